# revision 26
# baseline (speedup 1.0000x reference)
"""Trainium2 Bass kernel for nn_ExcEmbedding (ragged caption/image cosine sims).

v3 design (v1 ~90-92us, v2 ~65.5us):
  - fp8e4 row streams AND fp8e4 indicator matrices (host converts); bf16
    weights/elementwise; mixed-dtype matmuls (bf16/fp8); f32 PSUM.
  - Elementwise ops run on PAIRS of row tiles ([128, 2048]) to amortize
    the per-instruction overhead (ACT +352, DVE +58/151 element-times).
  - Cap phase is data-paced (no anti-hoist gating): ACT/DVE interleave
    cap tiles with img work as DMA chunks land. The broadcast trigger
    critical's entry snapshot still orders the trigger after my_vb.
  - Broadcast payload is fp8e4 carrying 16*v^T (the x16 keeps v's ~N(0,
    1/1024) entries out of fp8 subnormals); 32KB/dest halves the
    D2D-bound all-to-all vs bf16. Host epilogue divides the scales back
    (num,vg /16, q2 /256). vt2=(16v)^2 is computed by ACT Square (idle
    after the sigmoid), freeing the DVE tail.
  - rsqrt = ACT Sqrt table + DVE reciprocal_approx_fast (~51 ULP); no
    Newton-Raphson — the ~0.4% sqrt-table error is far inside the 2e-2
    budget (measured: no effect at 1.4e-4).
  - No mid-kernel barrier/clears: user sems are zeroed at NEFF init and
    by the previous run's postamble; the prelude AllGather (registered
    without any wait) initializes the NRT comm for D2D. The ~6us Q7
    remote_dma lib load is paid in an early critical during input DMA.
  - Final normalization epilogue runs on the HOST in f64 from shipped
    num/vg [64,256], q2 [32,256], cv [32,1024] per core.
"""

import os
import numpy as np
import ml_dtypes

import concourse.bass as bass
import concourse.bacc as bacc
import concourse.mybir as mybir
import concourse.tile as tile
from concourse.bass_utils import run_bass_kernel_spmd

F32 = mybir.dt.float32
BF16 = mybir.dt.bfloat16
AF = mybir.ActivationFunctionType
ALU = mybir.AluOpType

NCORES = 8
B = 256
R = 36
T = 64
D = 1024
DSQ = 128
M = B // NCORES          # 32 local captions / images per core
NI = M * R // 128        # 9 img row tiles of (128, D)
NC = M * T // 128        # 16 cap row tiles of (128, D)
KD = D // 128            # 8 d-blocks
SEG = KD * M             # 256 columns per (rank, stat) block
VSCALE = 16.0            # my_vb carries VSCALE*v^T in fp8e4

IMG_PAIRS = [(0, 2), (2, 4), (4, 6), (6, 8), (8, 9)]
CAP_PAIRS = [(0, 2), (2, 4), (4, 6), (6, 8), (8, 10), (10, 12), (12, 14),
             (14, 16)]
IMG_DVE_PAIRS = {2}      # pair indices whose leaky runs on DVE
CAP_DVE_PAIRS = {2, 5}

NOCOLL = os.environ.get("KV2_NOCOLL", "0") == "1"


def build_program(beta: float):
    nc = bacc.Bacc("TRN2", target_bir_lowering=False, debug=False,
                   num_devices=NCORES)

    IN_DT = mybir.dt.float8e4
    img_rows = nc.dram_tensor("img_rows", [NI * 128, D], IN_DT, kind="ExternalInput")
    cap_rows = nc.dram_tensor("cap_rows", [NC * 128, D], IN_DT, kind="ExternalInput")
    ei_t = nc.dram_tensor("ei_t", [128, NI * M], IN_DT, kind="ExternalInput")
    ec2_t = nc.dram_tensor("ec2_t", [128, NC * 2 * M], IN_DT, kind="ExternalInput")
    w_sq_t = nc.dram_tensor("w_sq_t", [128, D], BF16, kind="ExternalInput")
    w_ex_t = nc.dram_tensor("w_ex_t", [128, D], BF16, kind="ExternalInput")
    b_sq_t = nc.dram_tensor("b_sq_t", [DSQ, 1], F32, kind="ExternalInput")
    bexp_full = nc.dram_tensor("bexp_full", [128, SEG], F32, kind="ExternalInput")
    rlens = nc.dram_tensor("rlens", [M, 1], F32, kind="ExternalInput")
    idn32 = nc.dram_tensor("idn32", [M, M], F32, kind="ExternalInput")
    nvg_out = nc.dram_tensor("nvg_out", [2 * M, B], F32, kind="ExternalOutput")
    q2_out = nc.dram_tensor("q2_out", [M, B], F32, kind="ExternalOutput")
    cv_out = nc.dram_tensor("cv_out", [M, D], F32, kind="ExternalOutput")

    rsem = nc.alloc_semaphore(name="rsem")
    lsem = nc.alloc_semaphore(name="lsem")
    psem = nc.alloc_semaphore(name="psem")

    with tile.TileContext(nc) as tc:
        with (
            tc.tile_pool(name="consts", bufs=1) as consts,
            tc.tile_pool(name="xin", bufs=1) as xin,
            tc.tile_pool(name="yp", bufs=3) as yp,
            tc.tile_pool(name="y2p", bufs=3) as y2p,
            tc.tile_pool(name="ep", bufs=1) as ep,
            tc.tile_pool(name="smalls", bufs=1) as smalls,
            tc.tile_pool(name="tsb", bufs=1) as tsb,
            tc.tile_pool(name="psA", bufs=2, space="PSUM") as psA,
            tc.tile_pool(name="psT", bufs=1, space="PSUM") as psT,
            tc.tile_pool(name="psF", bufs=1, space="PSUM") as psF,
        ):
            # ---- input DMAs: first img chunk first (trigger path), then
            # the early-needed smalls, then the rest ----
            ximg = xin.tile([128, NI, D], IN_DT, name="ximg")
            nc.sync.dma_start(
                ximg[:, 0:2, :],
                img_rows[0:256, :].rearrange("(t p) d -> p t d", p=128))
            ei_sb = consts.tile([128, NI, M], IN_DT)
            nc.sync.dma_start(ei_sb[:], ei_t[:].rearrange("p (t c) -> p t c", t=NI))
            for j, e in ((2, 6), (6, 9)):
                nc.sync.dma_start(
                    ximg[:, j:e, :],
                    img_rows[128 * j:128 * e, :].rearrange(
                        "(t p) d -> p t d", p=128))
            idn_sb = consts.tile([M, M], F32)
            nc.sync.dma_start(idn_sb[:], idn32[:])
            # cap-side DMAs are emitted AFTER the trigger critical (below) so
            # its entry snapshot doesn't wait on them; the sync queue still
            # issues them right behind the img DMAs.
            xcap = xin.tile([128, NC, D], IN_DT, name="xcap")
            ec_sb = consts.tile([128, NC, 2 * M], IN_DT)
            rlens_sb = consts.tile([M, 1], F32)
            bsq_sb = consts.tile([DSQ, 1], F32)
            wsq_sb = consts.tile([128, D], BF16)
            wex_sb = consts.tile([128, D], BF16)
            bexp_sb = consts.tile([128, SEG], F32)

            # ---- dummy Sqrt pins the sqrt table set early ----
            dumm = smalls.tile([1, 1], F32, name="dumm")
            nc.vector.memset(dumm[:], 1.0)
            dum2 = smalls.tile([1, 1], F32, name="dum2")
            nc.scalar.activation(dum2[:], dumm[:], AF.Sqrt)

            vv = tsb.tile([128, NCORES * SEG + 2], IN_DT, name="vv")
            my_vb = tsb.tile([128, SEG], IN_DT, name="my_vb")
            if not NOCOLL:
                # Prelude AllGather registration (NRT comm init for the D2D
                # broadcast path) without any barrier wait instruction.
                nc._bir_kernel_barrier_sem_replica_groups.append(
                    set(range(NCORES)))
                # Early Pool-only critical: rank + the ~6us Q7 remote_dma
                # lib IRAM load, paid while the input DMAs stream.
                from concourse import library_config
                with tc.tile_critical(no_gpsimd_drain=True):
                    rank = nc.gpsimd.partition_id()
                    nc.gpsimd.load_library(library_config.remote_dma)

            # dep_b/dep_s: [128,1] bias=0 / scalar=0.1 APs produced from
            # my_vb. Threading them through the cap-phase elementwise ops
            # stops the scheduler from interleaving cap work into the img
            # ACT/DVE streams (that would delay the broadcast trigger).
            def leaky_square(x, n, on_act, dep_b=None, dep_s=None):
                # x: [128, n, D] slice; returns y, y2 as [128, n, D] bf16
                y = yp.tile([128, 2, D], BF16, name="y")[:, 0:n, :]
                if on_act:
                    nc.scalar.activation(y, x, AF.Prelu, alpha=0.1,
                                         bias=dep_b[:] if dep_b is not None
                                         else 0.0)
                else:
                    nc.vector.scalar_tensor_tensor(
                        y, x, dep_s[:] if dep_s is not None else 0.1, x,
                        op0=ALU.mult, op1=ALU.max)
                y2 = y2p.tile([128, 2, D], BF16, name="y2")[:, 0:n, :]
                nc.vector.tensor_tensor(y2, y, y, op=ALU.mult)
                return y, y2

            # ---- PE warm-up: ~3.5us of junk matmuls on ei_sb while the
            # input DMAs stream, so the HAM clock gate reaches 8/8 (2.4GHz)
            # before the real matmuls start; all later matmuls then run at
            # ~2x the cold rate as long as PE gaps stay under ~3.4us. ----
            # (warm-ups write into s12i; the img group's start=True clears
            # has_written, so the junk is fully overwritten)
            s12i = psA.tile([2 * M, D], F32, tag="acc", name="s12i")
            for w in range(15):
                nc.tensor.matmul(s12i[0:M, 0:64], ei_sb[:, 0, :],
                                 ei_sb[:, 0:2, :], skip_group_check=True)
            for pi, (a, b) in enumerate(IMG_PAIRS):
                n = b - a
                y, y2 = leaky_square(ximg[:, a:b, :], n,
                                     pi not in IMG_DVE_PAIRS)
                for t in range(a, b):
                    for h in range(2):
                        cs = slice(512 * h, 512 * (h + 1))
                        nc.tensor.matmul(s12i[0:M, cs], ei_sb[:, t, :],
                                         y[:, t - a, cs],
                                         start=(t == 0), stop=(t == NI - 1),
                                         skip_group_check=True)
                        nc.tensor.matmul(s12i[M:2 * M, cs], ei_sb[:, t, :],
                                         y2[:, t - a, cs],
                                         start=(t == 0), stop=(t == NI - 1),
                                         skip_group_check=True)

            # ---- img epilogue (by halves) + transpose 16*v^T -> fp8 ----
            v = smalls.tile([M, D], F32, name="v")
            vps = psT.tile([128, SEG], F32, tag="t", name="vps")
            for h in range(2):
                cs = slice(512 * h, 512 * (h + 1))
                sq = ep.tile([M, 512], F32, name=f"sqi{h}")
                nc.scalar.activation(sq[:], s12i[M:2 * M, cs], AF.Sqrt)
                r1 = ep.tile([M, 512], F32, name=f"ri{h}")
                nc.vector.reciprocal_approx_fast(r1[:], sq[:])
                nc.vector.scalar_tensor_tensor(v[:, cs], s12i[0:M, cs],
                                               1.0 / R, r1[:],
                                               op0=ALU.mult, op1=ALU.mult)
                for k in range(4 * h, 4 * (h + 1)):
                    nc.tensor.transpose(vps[:, M * k:M * (k + 1)],
                                        v[:, 128 * k:128 * (k + 1)], idn_sb[:])
                nc.vector.tensor_scalar_mul(my_vb[:, 128 * h:128 * (h + 1)],
                                            vps[:, 128 * h:128 * (h + 1)],
                                            VSCALE)

            zb = smalls.tile([128, 1], F32, name="zb")
            nc.vector.tensor_scalar_mul(zb[:], vps[:, 0:1], 0.0)
            pt1 = smalls.tile([128, 1], F32, name="pt1")
            nc.vector.tensor_scalar_add(pt1[:], zb[:], 0.1)

            # ---- trigger critical: Pool-only; the descgen's my_vb read puts
            # the img-epilogue dependency on this critical's entry snapshot.
            if NOCOLL:
                for g in range(NCORES):
                    nc.vector.tensor_copy(vv[:, SEG * g:SEG * (g + 1)],
                                          my_vb[:])
            else:
                with tc.tile_critical(no_gpsimd_drain=True):
                    nc.gpsimd.remote_dma_broadcast(
                        vv[:, bass.ds(rank * SEG, SEG)], my_vb[:],
                        remote_sem=rsem, local_sem=lsem,
                        rdests=[(0, j) for j in range(NCORES)],
                    ).then_inc(psem, 1)
                    nc.gpsimd.wait_ge(psem, 1)
                    nc.gpsimd.trigger_dma(count=1)

            # ---- cap-side DMAs (see note above) ----
            nc.sync.dma_start(ec_sb[:], ec2_t[:].rearrange("p (t c) -> p t c", t=NC))
            for j, e in ((0, 4), (4, 8), (8, 12), (12, 16)):
                nc.sync.dma_start(
                    xcap[:, j:e, :],
                    cap_rows[128 * j:128 * e, :].rearrange(
                        "(t p) d -> p t d", p=128))
            nc.sync.dma_start(rlens_sb[:], rlens[:])
            nc.sync.dma_start(bsq_sb[:], b_sq_t[:])
            nc.sync.dma_start(wsq_sb[:], w_sq_t[:])
            nc.sync.dma_start(wex_sb[:], w_ex_t[:])
            nc.sync.dma_start(bexp_sb[:], bexp_full[:])

            # ---- cap phase (gated behind the img epilogue via zb/pt1) ----
            s12c = psA.tile([2 * M, D], F32, tag="acc", name="s12c")
            for pi, (a, b) in enumerate(CAP_PAIRS):
                n = b - a
                yc, yc2 = leaky_square(xcap[:, a:b, :], n,
                                       pi not in CAP_DVE_PAIRS,
                                       dep_b=zb, dep_s=pt1)
                for t in range(a, b):
                    for h in range(2):
                        cs = slice(512 * h, 512 * (h + 1))
                        nc.tensor.matmul(s12c[0:M, cs], ec_sb[:, t, 0:M],
                                         yc[:, t - a, cs],
                                         start=(t == 0), stop=(t == NC - 1),
                                         skip_group_check=True)
                        nc.tensor.matmul(s12c[M:2 * M, cs],
                                         ec_sb[:, t, M:2 * M],
                                         yc2[:, t - a, cs],
                                         start=(t == 0), stop=(t == NC - 1),
                                         skip_group_check=True)

            # ---- cap epilogue (by halves) + transpose cv -> cvt bf16 ----
            cv = smalls.tile([M, D], F32, name="cv")
            cvps = psT.tile([128, SEG], F32, tag="t2", name="cvps")
            cvt = tsb.tile([128, SEG], BF16, name="cvt")
            for h in range(2):
                cs = slice(512 * h, 512 * (h + 1))
                sq = ep.tile([M, 512], F32, name=f"sqc{h}")
                nc.scalar.activation(sq[:], s12c[M:2 * M, cs], AF.Sqrt)
                r1 = ep.tile([M, 512], F32, name=f"rc{h}")
                nc.vector.reciprocal_approx_fast(r1[:], sq[:])
                nc.vector.scalar_tensor_tensor(cv[:, cs], s12c[0:M, cs],
                                               rlens_sb[:], r1[:],
                                               op0=ALU.mult, op1=ALU.mult)
                for k in range(4 * h, 4 * (h + 1)):
                    nc.tensor.transpose(cvps[:, M * k:M * (k + 1)],
                                        cv[:, 128 * k:128 * (k + 1)], idn_sb[:])
                nc.vector.tensor_copy(cvt[:, 128 * h:128 * (h + 1)],
                                      cvps[:, 128 * h:128 * (h + 1)])
            nc.sync.dma_start(cv_out[:], cv[:])
            # Dummy sigmoid right after the last Sqrt: hoists the ~2.6us
            # sigmoid table-set load off the tail (overlaps gate matmuls).
            dum3 = smalls.tile([1, 1], F32, name="dum3")
            nc.scalar.activation(dum3[:], dumm[:], AF.Sigmoid)

            # ---- gate ----
            ht_ps = psF.tile([DSQ, M], F32, tag="f", name="ht_ps")
            for k in range(KD):
                nc.tensor.matmul(ht_ps[:], wsq_sb[:, 128 * k:128 * (k + 1)],
                                 cvt[:, M * k:M * (k + 1)],
                                 start=(k == 0), stop=(k == KD - 1),
                                 skip_group_check=True)
            ht = tsb.tile([DSQ, M], BF16, name="ht")
            nc.scalar.activation(ht[:], ht_ps[:], AF.Relu, bias=bsq_sb[:])

            gps = psT.tile([128, SEG], F32, tag="g", name="gps")
            for k in range(KD):
                nc.tensor.matmul(gps[:, M * k:M * (k + 1)],
                                 wex_sb[:, 128 * k:128 * (k + 1)], ht[:],
                                 skip_group_check=True)
            gpb = tsb.tile([128, SEG], F32, name="gpb")
            nc.vector.tensor_tensor(gpb[:], gps[:], bexp_sb[:], op=ALU.add)
            # sigmoid writes gt DIRECTLY into agt's odd 32-col blocks
            # (strided dst), so no separate copy pass is needed; at = gt*cvt
            # fills the even blocks with one strided TT.
            agt = tsb.tile([128, 2 * SEG], BF16, name="agt")
            agt4 = agt[:].rearrange("p (k two c) -> p k two c", k=KD, two=2)
            cvt4 = cvt[:].rearrange("p (k c) -> p k c", k=KD)
            nc.scalar.activation(agt4[:, :, 1, :], gpb[:], AF.Sigmoid)
            g2t = tsb.tile([128, SEG], BF16, name="g2t")
            nc.vector.tensor_tensor(g2t[:].rearrange("p (k c) -> p k c", k=KD),
                                    agt4[:, :, 1, :], agt4[:, :, 1, :],
                                    op=ALU.mult)
            nc.vector.tensor_tensor(agt4[:, :, 0, :], agt4[:, :, 1, :],
                                    cvt4, op=ALU.mult)

            # ---- critical B: wait for the gathered payload; the memset of a
            # padding column of vv gives the finals a RAW dep on it ----
            if not NOCOLL:
                with tc.tile_critical(no_gpsimd_drain=True):
                    nc.gpsimd.wait_ge(rsem, NCORES * 2)
                    nc.gpsimd.memset(vv[0:1, NCORES * SEG:NCORES * SEG + 2], 0)

            # ---- finals: [num|vg] packed [2M, B], q2 [M, B] ----
            # vt2 = (16v)^2 on ACT (idle after the sigmoid); the tiny copy
            # chains it behind agt in the ACT queue so the scheduler cannot
            # hoist the landing-dependent square ahead of the gate chain.
            vt2 = tsb.tile([128, NCORES * SEG], BF16, name="vt2")
            nc.scalar.copy(vt2[0:1, 0:2], agt[0:1, 0:2])
            nc.scalar.activation(vt2[:], vv[:, 0:NCORES * SEG], AF.Square)
            vv4 = vv[:, 0:NCORES * SEG].rearrange("p (g k c) -> p g k c",
                                                  g=NCORES, k=KD)
            vt24 = vt2[:].rearrange("p (g k c) -> p g k c", g=NCORES, k=KD)
            nvg_ps = psF.tile([2 * M, 512], F32, tag="f", name="nvg_ps")
            q2_ps = psF.tile([M, 512], F32, tag="f", name="q2_ps")
            for k in range(KD):
                nc.tensor.matmul(nvg_ps[:, 0:B], agt[:, 2 * M * k:2 * M * (k + 1)],
                                 vv4[:, :, k, :],
                                 start=(k == 0), stop=(k == KD - 1),
                                 skip_group_check=True)
            nvgs = smalls.tile([2 * M, B], F32, name="nvgs")
            nc.scalar.copy(nvgs[:], nvg_ps[:, 0:B])
            nc.sync.dma_start(nvg_out[:], nvgs[:])
            for k in range(KD):
                nc.tensor.matmul(q2_ps[:, 0:B], g2t[:, M * k:M * (k + 1)],
                                 vt24[:, :, k, :],
                                 start=(k == 0), stop=(k == KD - 1),
                                 skip_group_check=True)
            qsb = smalls.tile([M, B], F32, name="qsb")
            nc.scalar.copy(qsb[:], q2_ps[:, 0:B])
            nc.sync.dma_start(q2_out[:], qsb[:])

    nc.compile()
    return nc


_PROG_CACHE: dict = {}


def get_program(beta: float):
    if beta not in _PROG_CACHE:
        _PROG_CACHE[beta] = build_program(beta)
    return _PROG_CACHE[beta]


def make_in_maps(img_embed, cap_embed, lens, W_sq, b_sq, W_ex, b_ex):
    bf = ml_dtypes.bfloat16
    f8 = ml_dtypes.float8_e4m3
    img_bf = np.ascontiguousarray(img_embed, dtype=np.float32).astype(f8)
    cap_bf = np.ascontiguousarray(cap_embed, dtype=np.float32).astype(f8)
    lens_i = np.asarray(lens).astype(np.int64)

    # W_sq (D, DSQ) -> [128, KD*128]: w_sq_t[p, 128k+j] = W_sq[128k+p, j]
    w_sq_np = np.asarray(W_sq, dtype=np.float32).astype(bf)
    w_sq_t_np = np.ascontiguousarray(
        w_sq_np.reshape(KD, 128, DSQ).transpose(1, 0, 2).reshape(128, D))
    w_ex_t_np = np.ascontiguousarray(np.asarray(W_ex, dtype=np.float32).astype(bf))
    b_sq_np = np.ascontiguousarray(
        np.asarray(b_sq, dtype=np.float32).reshape(DSQ, 1))
    # bexp_full[p, M*k + c] = +b_ex[128k + p]
    bex = np.asarray(b_ex, dtype=np.float32)
    bexp_np = np.ascontiguousarray(
        np.repeat(bex.reshape(KD, 128).T, M, axis=1).reshape(128, SEG))
    idn_np = np.eye(M, dtype=np.float32)

    ei_np = np.zeros((NI * 128, M), dtype=np.float32)
    rows_i = np.arange(M * R)
    ei_np[rows_i, rows_i // R] = 1.0
    ei_t_np = ei_np.reshape(NI, 128, M).transpose(1, 0, 2).reshape(
        128, NI * M).astype(f8)

    in_maps = []
    for j in range(NCORES):
        sl = slice(M * j, M * (j + 1))
        lens_local = lens_i[sl]
        ec2_np = np.zeros((M * T, 2 * M), dtype=np.float32)
        rows = np.arange(M * T)
        cidx = rows // T
        tidx = rows % T
        ec2_np[rows, M + cidx] = 1.0
        keep = tidx < lens_local[cidx]
        ec2_np[rows[keep], cidx[keep]] = 1.0
        ec2_t_np = ec2_np.reshape(NC, 128, 2 * M).transpose(1, 0, 2).reshape(
            128, NC * 2 * M).astype(f8)
        rlens_np = (1.0 / lens_local.astype(np.float64)).astype(
            np.float32).reshape(M, 1)

        in_maps.append({
            "img_rows": np.ascontiguousarray(img_bf[sl].reshape(M * R, D)),
            "cap_rows": np.ascontiguousarray(cap_bf[sl].reshape(M * T, D)),
            "ei_t": np.ascontiguousarray(ei_t_np),
            "ec2_t": np.ascontiguousarray(ec2_t_np),
            "w_sq_t": w_sq_t_np,
            "w_ex_t": w_ex_t_np,
            "b_sq_t": b_sq_np,
            "bexp_full": bexp_np,
            "rlens": rlens_np,
            "idn32": idn_np,
        })
    return in_maps


LAST_RESULT = None
EPS = 1e-8


def kernel(img_embed, cap_embed, lens, W_sq, b_sq, W_ex, b_ex, beta, beta1):
    global LAST_RESULT
    beta_f = float(np.asarray(beta).reshape(-1)[0])
    nc = get_program(beta_f)
    in_maps = make_in_maps(img_embed, cap_embed, lens, W_sq, b_sq, W_ex, b_ex)
    res = run_bass_kernel_spmd(nc, in_maps, core_ids=list(range(NCORES)))
    LAST_RESULT = res
    sims = np.empty((B, B), dtype=np.float32)
    for j in range(NCORES):
        r = res.results[j]
        nvg = r["nvg_out"].astype(np.float64)   # (2M, B)
        num = nvg[0:M] / VSCALE
        vg = nvg[M:2 * M] / VSCALE
        q2 = r["q2_out"].astype(np.float64) / (VSCALE * VSCALE)
        cv = r["cv_out"].astype(np.float64)     # (M, D)
        rn = 1.0 / (np.sqrt((cv * cv).sum(axis=1, keepdims=True)) + EPS)
        bias = beta_f * cv.sum(axis=1, keepdims=True) * rn
        denom = np.sqrt(q2 + 2.0 * beta_f * vg + beta_f * beta_f * D) + EPS
        simst = (num * rn + bias) / denom       # (M, B) = sims[:, block].T
        sims[:, M * j:M * (j + 1)] = simst.T.astype(np.float32)
    return sims
